# revision 25
# baseline (speedup 1.0000x reference)
"""Distributed Trainium2 kernel for CustomMultiHeadAttentionStoich (v4).

Sharding (8 cores): core c = (batch b=c//4, slice p=c%4); each core owns
512 queries and all 16 heads for them.

v4 structure:
 - K^T projection row-interleaved (host permutes Wk^T columns); V
   projection key-sliced. Four 512KB-contribution AllGathers (K01, V01,
   K23, V23) reassemble full K^T / V per 8-head half, triggered as soon
   as each projection chunk lands; a tiny warmup AllGather issued at
   t~0 absorbs the first-collective ramp.
 - Input DMAs are split into ~1MB pieces and spread across the SP and
   Activation trigger engines in need-order (K path first), so no
   single hardware queue serializes a 4MB tensor.
 - Attention is globally software-pipelined: the AV matmul for score
   pair k issues while score pair k+3 is computed, ACROSS head
   boundaries, so the PE alternates scores/AV continuously and the
   exp latency (Activation engine) never stalls it.
 - Softmax exp at double width ([128,1024] PSUM -> one ACTIVATE per two
   key chunks). Denominators ride a locally-memset ones-column of the
   AV stationary; per-pair reciprocals; the PE-side normalization
   broadcast + out-projection accumulation for a finished head pair are
   deferred into the middle of the NEXT head's score stream.
 - Out-projection streamed per head pair into SBUF fp32 accumulators
   (vector adds); after the last head only one pair's work + output
   DMAs remain.
"""

import sys

sys.path.insert(0, "/opt/trn_rl_repo")

import numpy as np
import ml_dtypes

BF = ml_dtypes.bfloat16

B, T, D, H, DH = 2, 2048, 1024, 16, 64
NCORES = 8
TQ = 512  # queries per core
R = 16  # SVD rank per clamp-kernel half
AUG = 2 * R
CP = DH + AUG  # contraction rows for the scores matmul
NGRID = 1024  # SVD grid
KC = T // 128  # 16 key chunks

_state = {}


def _features():
    """Rank-R SVD features of rc(x,y)=clip(x-y,0,0.2) on [0,1]^2."""
    if "grid" not in _state:
        g = (np.arange(NGRID) + 0.5) / NGRID
        M = np.clip(g[:, None] - g[None, :], 0.0, 0.2)
        U, S, Vt = np.linalg.svd(M, full_matrices=False)
        sc = np.sqrt(S[:R] * NGRID)
        _state["grid"] = g
        _state["phi"] = (U[:, :R] * sc).astype(np.float64)
        _state["psi"] = (Vt[:R].T * sc).astype(np.float64)
    return _state["grid"], _state["phi"], _state["psi"]


def _ev(tab, x):
    g = _state["grid"]
    return np.stack([np.interp(x, g, tab[:, j]) for j in range(R)])


def _build():
    if "nc" in _state:
        return _state["nc"]

    import concourse.bass as bass
    import concourse.mybir as mybir
    import concourse.tile as tile
    from concourse import bacc

    dt = mybir.dt
    ts = bass.ts
    ds = bass.ds

    nc = bacc.Bacc(
        "TRN2",
        target_bir_lowering=False,
        debug=False,
        num_devices=NCORES,
    )

    # ---- kernel I/O (per-core shards; host pre-slices) ----
    xqT = nc.dram_tensor("xqT", [D, TQ], dt.bfloat16, kind="ExternalInput").ap()
    xkT = nc.dram_tensor("xkT", [D, T], dt.bfloat16, kind="ExternalInput").ap()
    xvT = nc.dram_tensor("xvT", [D, TQ], dt.bfloat16, kind="ExternalInput").ap()
    wqT = nc.dram_tensor("wqT", [D, D], dt.bfloat16, kind="ExternalInput").ap()
    wkP = nc.dram_tensor("wkP", [D, 256], dt.bfloat16, kind="ExternalInput").ap()
    wvT = nc.dram_tensor("wvT", [D, D], dt.bfloat16, kind="ExternalInput").ap()
    woT = nc.dram_tensor("woT", [D, D], dt.bfloat16, kind="ExternalInput").ap()
    bqE = nc.dram_tensor("bq", [128, 8], dt.float32, kind="ExternalInput").ap()
    bkE = nc.dram_tensor("bkP", [128, 2], dt.float32, kind="ExternalInput").ap()
    bvE = nc.dram_tensor("bv", [1, D], dt.bfloat16, kind="ExternalInput").ap()
    boE = nc.dram_tensor("bo", [1, D], dt.bfloat16, kind="ExternalInput").ap()
    kfE = nc.dram_tensor("kfeat", [AUG, T], dt.bfloat16, kind="ExternalInput").ap()
    qfE = nc.dram_tensor("qfeat", [H * AUG, TQ], dt.bfloat16, kind="ExternalInput").ap()
    selE = nc.dram_tensor("sel2", [2, 128], dt.bfloat16, kind="ExternalInput").ap()
    outE = nc.dram_tensor("out", [TQ, D], dt.float32, kind="ExternalOutput").ap()

    Exp = mybir.ActivationFunctionType.Exp
    RG = [[0, 1, 2, 3], [4, 5, 6, 7]]
    Bypass = mybir.AluOpType.bypass

    with tile.TileContext(nc) as tc:
        with (
            tc.tile_pool(name="dram", bufs=1, space="DRAM") as dram,
            tc.tile_pool(name="consts", bufs=1) as consts,
            tc.tile_pool(name="ehat", bufs=6) as ep,
            tc.tile_pool(name="stage", bufs=6) as stp,
            tc.tile_pool(name="rr", bufs=2) as rrp,
            tc.tile_pool(name="dn", bufs=2) as dnp,
            tc.tile_pool(name="dp", bufs=2) as dpp,
            tc.tile_pool(name="db", bufs=2) as dbp,
            tc.tile_pool(name="psS", bufs=2, space="PSUM") as psS,
            tc.tile_pool(name="psAV", bufs=2, space="PSUM") as psAV,
            tc.tile_pool(name="psY", bufs=2, space="PSUM") as psY,
        ):
            # DRAM staging for the chunked collectives
            warm_in = dram.tile([1, 64], dt.bfloat16, tag="wi", name="wi")
            warm_out = dram.tile([4, 64], dt.bfloat16, tag="wo2", name="wo2")
            kT_part = [
                dram.tile([128, T], dt.bfloat16, tag=f"kp{i}", name=f"kp{i}")
                for i in range(2)
            ]
            kT_g = [
                dram.tile([512, T], dt.bfloat16, tag=f"kg{i}", name=f"kg{i}")
                for i in range(2)
            ]
            v_part = [
                dram.tile([TQ, 512], dt.bfloat16, tag=f"vp{i}", name=f"vp{i}")
                for i in range(2)
            ]
            v_g = [
                dram.tile([T, 512], dt.bfloat16, tag=f"vg{i}", name=f"vg{i}")
                for i in range(2)
            ]

            # ---- resident constants ----
            wk_sb = consts.tile([128, 8, 256], dt.bfloat16, tag="wk", name="wk")
            xk_sb = consts.tile([128, 8, T], dt.bfloat16, tag="xk", name="xk")
            wv_sb = consts.tile([128, 8, D], dt.bfloat16, tag="wv", name="wv")
            xv_sb = consts.tile([128, 8, TQ], dt.bfloat16, tag="xv", name="xv")
            wq_sb = consts.tile([128, 8, D], dt.bfloat16, tag="wq", name="wq")
            xq_sb = consts.tile([128, 8, TQ], dt.bfloat16, tag="xq", name="xq")
            wo_sb = consts.tile([128, 8, D], dt.bfloat16, tag="wo", name="wo")
            bq_sb = consts.tile([128, 8], dt.float32, tag="bq", name="bq")
            bk_sb = consts.tile([128, 2], dt.float32, tag="bk", name="bk")
            bv_sb = consts.tile([1, D], dt.bfloat16, tag="bv", name="bv")
            bo_sb = consts.tile([1, D], dt.bfloat16, tag="bo", name="bo")
            ones_sb = consts.tile([1, 128], dt.bfloat16, tag="ones", name="ones")
            sel2 = consts.tile([2, 128], dt.bfloat16, tag="sel2", name="sel2")
            yacc = [
                consts.tile([128, 512], dt.float32, tag=f"y{i}", name=f"y{i}")
                for i in range(8)
            ]
            qat = [
                consts.tile([CP, TQ], dt.bfloat16, tag=f"qat{h}", name=f"qat{h}")
                for h in range(H)
            ]
            aot = [
                consts.tile([128, TQ], dt.bfloat16, tag=f"aot{pr}", name=f"aot{pr}")
                for pr in range(8)
            ]
            kat_bufs = [
                consts.tile([CP, T], dt.bfloat16, tag=f"kat{i}", name=f"kat{i}")
                for i in range(3)
            ]
            vtl_bufs = [
                consts.tile([128, KC, 65], dt.bfloat16, tag=f"vtl{i}", name=f"vtl{i}")
                for i in range(3)
            ]

            # warmup collective: pay the first-CC ramp before real AGs
            nc.sync.dma_start(out=warm_in, in_=selE[0:1, 0:64])
            nc.gpsimd.collective_compute(
                "AllGather",
                Bypass,
                ins=[warm_in.opt()],
                outs=[warm_out.opt()],
                replica_groups=RG,
            )

            # K/V-path inputs on the SP trigger engine, split for queue
            # parallelism, in need-order
            for i in range(4):
                nc.sync.dma_start(
                    out=wk_sb[:, ts(i, 2), :],
                    in_=wkP.rearrange("(a p) m -> p a m", p=128)[:, ts(i, 2), :],
                )
            for i in range(4):
                nc.sync.dma_start(
                    out=xk_sb[:, :, ts(i, 512)],
                    in_=xkT[:, ts(i, 512)].rearrange("(a p) m -> p a m", p=128),
                )
            for i in range(2):
                nc.sync.dma_start(
                    out=xv_sb[:, ts(i, 4), :],
                    in_=xvT.rearrange("(a p) m -> p a m", p=128)[:, ts(i, 4), :],
                )
            for i in range(4):
                nc.sync.dma_start(
                    out=wv_sb[:, ts(i, 2), :],
                    in_=wvT.rearrange("(a p) m -> p a m", p=128)[:, ts(i, 2), :],
                )
            # Q-path + small inputs on the Activation trigger engine
            nc.scalar.dma_start(out=bk_sb, in_=bkE)
            nc.scalar.dma_start(out=bv_sb, in_=bvE)
            for i in range(2):
                nc.scalar.dma_start(
                    out=xq_sb[:, ts(i, 4), :],
                    in_=xqT.rearrange("(a p) m -> p a m", p=128)[:, ts(i, 4), :],
                )
            for i in range(4):
                nc.scalar.dma_start(
                    out=wq_sb[:, ts(i, 2), :],
                    in_=wqT.rearrange("(a p) m -> p a m", p=128)[:, ts(i, 2), :],
                )
            nc.scalar.dma_start(out=bq_sb, in_=bqE)
            nc.scalar.dma_start(out=sel2, in_=selE)
            nc.scalar.dma_start(out=bo_sb, in_=boE)
            for i in range(3):
                nc.scalar.dma_start(out=kat_bufs[i][DH:CP, :], in_=kfE)
            for h in range(H):
                nc.scalar.dma_start(out=qat[h][DH:CP, :], in_=qfE[ds(AUG * h, AUG), :])
            nc.vector.memset(ones_sb, 1.0)
            for i in range(3):
                nc.vector.memset(vtl_bufs[i][:, :, 64:65], 1.0)

            # ---- K^T projection (row-interleaved shard) -> AGK chunks ----
            def k_proj(gpair):
                for nch in range(4):
                    ps = psY.tile([128, TQ], dt.float32, tag="y", name="mmk")
                    for kc in range(8):
                        nc.tensor.matmul(
                            ps,
                            lhsT=wk_sb[:, kc, ts(gpair, 128)],
                            rhs=xk_sb[:, kc, ts(nch, 512)],
                            start=(kc == 0),
                            stop=(kc == 7),
                        )
                    stg = stp.tile([128, TQ], dt.bfloat16, tag="stg", name="kstg")
                    nc.vector.tensor_scalar_add(stg, ps, bk_sb[:, gpair : gpair + 1])
                    nc.sync.dma_start(out=kT_part[gpair][:, ts(nch, 512)], in_=stg)
                nc.gpsimd.collective_compute(
                    "AllGather",
                    Bypass,
                    ins=[kT_part[gpair].opt()],
                    outs=[kT_g[gpair].opt()],
                    replica_groups=RG,
                )

            # ---- V projection (key-slice shard) -> AGV chunks ----
            def v_proj(half):
                for tc_i in range(4):
                    ps = psY.tile([128, TQ], dt.float32, tag="y", name="mmv")
                    for kc in range(8):
                        nc.tensor.matmul(
                            ps,
                            lhsT=xv_sb[:, kc, ts(tc_i, 128)],
                            rhs=wv_sb[:, kc, ts(half, 512)],
                            start=(kc == 0),
                            stop=False,
                        )
                    nc.tensor.matmul(
                        ps,
                        lhsT=ones_sb[:, :],
                        rhs=bv_sb[:, ts(half, 512)],
                        start=False,
                        stop=True,
                    )
                    stg = stp.tile([128, TQ], dt.bfloat16, tag="stg", name="vstg")
                    nc.vector.tensor_copy(stg, ps)
                    nc.sync.dma_start(out=v_part[half][ts(tc_i, 128), :], in_=stg)
                nc.gpsimd.collective_compute(
                    "AllGather",
                    Bypass,
                    ins=[v_part[half].opt()],
                    outs=[v_g[half].opt()],
                    replica_groups=RG,
                )

            k_proj(0)
            v_proj(0)
            k_proj(1)
            v_proj(1)

            # prefetch attention tiles for heads 0/1 ahead of Q-proj stage DMAs
            def head_loads(h):
                g2, j = h // 8, h % 4
                kat = kat_bufs[h % 3]
                vtl = vtl_bufs[h % 3]
                krow = 128 * j + 64 * ((h // 4) % 2)
                for i in range(2):
                    nc.sync.dma_start(
                        out=kat[0:DH, ts(i, 1024)],
                        in_=kT_g[g2][ds(krow, DH), ts(i, 1024)],
                    )
                nc.gpsimd.dma_start(
                    out=vtl[:, :, 0:64],
                    in_=v_g[g2][:, ds(DH * (h % 8), DH)].rearrange(
                        "(a p) m -> p a m", p=128
                    ),
                )

            head_loads(0)
            head_loads(1)

            # ---- Q projection ----
            for dc in range(8):
                ps = psY.tile([128, TQ], dt.float32, tag="y", name="mmq")
                for kc in range(8):
                    nc.tensor.matmul(
                        ps,
                        lhsT=wq_sb[:, kc, ts(dc, 128)],
                        rhs=xq_sb[:, kc, :],
                        start=(kc == 0),
                        stop=(kc == 7),
                    )
                stg = stp.tile([128, TQ], dt.bfloat16, tag="stg", name="qstg")
                nc.vector.tensor_scalar_add(stg, ps, bq_sb[:, dc : dc + 1])
                nc.gpsimd.dma_start(out=qat[2 * dc][0:DH, :], in_=stg[0:DH, :])
                nc.gpsimd.dma_start(out=qat[2 * dc + 1][0:DH, :], in_=stg[DH:128, :])

            # ---- attention: global scores/AV software pipeline ----
            pending_tail = []
            pair_state = {}

            def emit_pair_tail(pr):
                """PE-side norm + out-proj accumulation for finished pair."""
                dbf = pair_state.pop(pr)
                ps_r = psS.tile([128, 1024], dt.float32, tag="s", name="r")
                nc.tensor.matmul(
                    ps_r[:, 0:TQ], lhsT=sel2, rhs=dbf, start=True, stop=True
                )
                nc.vector.tensor_mul(aot[pr], aot[pr], ps_r[:, 0:TQ])
                for qc in range(4):
                    for mc in range(2):
                        ti = 2 * qc + mc
                        ps_y = psY.tile([128, TQ], dt.float32, tag="y", name="mmy")
                        nc.tensor.matmul(
                            ps_y,
                            lhsT=aot[pr][:, ts(qc, 128)],
                            rhs=wo_sb[:, pr, ds(512 * mc, 512)],
                            start=True,
                            stop=(pr != 0),
                        )
                        if pr == 0:
                            nc.tensor.matmul(
                                ps_y,
                                lhsT=ones_sb[:, :],
                                rhs=bo_sb[:, ds(512 * mc, 512)],
                                start=False,
                                stop=True,
                            )
                            nc.vector.tensor_copy(yacc[ti], ps_y)
                        else:
                            nc.vector.tensor_add(yacc[ti], yacc[ti], ps_y)

            def finish_head(h, ps_av):
                pr = h // 2
                if h % 2 == 0:
                    nc.vector.tensor_copy(aot[pr][0:DH, :], ps_av[0:DH, :])
                    dpair = dpp.tile([2, TQ], dt.float32, tag="dpr", name="dpr")
                    pair_state[("dp", pr)] = dpair
                else:
                    nc.vector.tensor_copy(aot[pr][DH:128, :], ps_av[0:DH, :])
                    dpair = pair_state[("dp", pr)]
                dstage = dnp.tile([128, TQ], dt.float32, tag="dst", name="dst")
                nc.vector.tensor_copy(dstage[DH : DH + 1, :], ps_av[DH : DH + 1, :])
                nc.gpsimd.dma_start(
                    out=dpair[ds(h % 2, 1), :], in_=dstage[DH : DH + 1, :]
                )
                if h % 2 == 1:
                    del pair_state[("dp", pr)]
                    nc.vector.reciprocal_approx_fast(dpair, dpair)
                    dbf = dbp.tile([2, TQ], dt.bfloat16, tag="dbf", name="dbf")
                    nc.vector.tensor_copy(dbf, dpair)
                    pair_state[pr] = dbf
                    pending_tail.append(lambda pr=pr: emit_pair_tail(pr))

            def make_av(h, kp, ps_av, vtl, eh):
                def emit():
                    for s in range(2):
                        kc = 2 * kp + s
                        nc.tensor.matmul(
                            ps_av[0:65, :],
                            lhsT=vtl[:, kc, :],
                            rhs=eh[:, ts(s, 512)],
                            start=(kc == 0),
                            stop=(kc == KC - 1),
                        )
                    if kp == 7:
                        finish_head(h, ps_av)

                return emit

            avq = []
            for h in range(H):
                kat = kat_bufs[h % 3]
                vtl = vtl_bufs[h % 3]
                if h >= 2:
                    head_loads(h)
                if h == 1:
                    for i in range(4):
                        nc.scalar.dma_start(
                            out=wo_sb[:, ts(i, 2), :],
                            in_=woT.rearrange("(a p) m -> p a m", p=128)[
                                :, ts(i, 2), :
                            ],
                        )
                ps_av = psAV.tile([128, TQ], dt.float32, tag="av", name="av")
                for kp in range(8):
                    ps2 = psS.tile([128, 1024], dt.float32, tag="s", name="s")
                    for s in range(2):
                        nc.tensor.matmul(
                            ps2[:, ts(s, 512)],
                            lhsT=kat[:, ts(2 * kp + s, 128)],
                            rhs=qat[h],
                            start=True,
                            stop=True,
                        )
                    eh = ep.tile([128, 1024], dt.bfloat16, tag="ehat", name="ehat")
                    nc.scalar.activation(eh, ps2, Exp)
                    avq.append(make_av(h, kp, ps_av, vtl, eh))
                    if kp == 7 and pending_tail:
                        pending_tail.pop(0)()
                    while len(avq) > 5:
                        avq.pop(0)()
            while avq:
                avq.pop(0)()
            while pending_tail:
                pending_tail.pop(0)()

            # ---- output DMA ----
            for qc in range(4):
                for mc in range(2):
                    nc.sync.dma_start(
                        out=outE[ts(qc, 128), ts(mc, 512)], in_=yacc[2 * qc + mc]
                    )

    nc.compile()
    _state["nc"] = nc
    return nc


def _make_in_maps(inputs):
    _features()
    if "sel2" not in _state:
        s = np.zeros((2, 128), BF)
        s[0, 0:64] = 1
        s[1, 64:128] = 1
        _state["sel2"] = s
    gs = float(np.float32(inputs["gamma"])) * DH ** -0.5
    delta = float(np.float32(inputs["delta"]))
    ap_ = np.asarray(inputs["alpha_pos"], np.float64)
    an_ = np.asarray(inputs["alpha_neg"], np.float64)

    wqTh = (np.asarray(inputs["Wq"], np.float64).T * gs).astype(BF)
    bq_f = (np.asarray(inputs["bq"], np.float64) * gs).astype(np.float32)
    bqh = np.ascontiguousarray(bq_f.reshape(8, 128).T)
    wkT_full = np.ascontiguousarray(np.asarray(inputs["Wk"]).T)
    bk_full = np.asarray(inputs["bk"], np.float32)
    wvTh = np.ascontiguousarray(np.asarray(inputs["Wv"]).T).astype(BF)
    bvh = np.asarray(inputs["bv"], np.float32)[None, :].astype(BF)
    woTh = np.ascontiguousarray(np.asarray(inputs["Wo"]).T).astype(BF)
    boh = np.asarray(inputs["bo"], np.float32)[None, :].astype(BF)

    phi, psi = _state["phi"], _state["psi"]
    frac = np.asarray(inputs["frac"], np.float64)

    in_maps = []
    for c in range(NCORES):
        b, p = c // 4, c % 4
        perm = np.concatenate(
            [np.arange(256 * g + 64 * p, 256 * g + 64 * p + 64) for g in range(4)]
        )
        fb = frac[b]
        fq = fb[TQ * p : TQ * (p + 1)]
        kfeat = np.concatenate([_ev(phi, fb), _ev(psi, fb)], 0).astype(BF)
        qfeat = np.zeros((H * AUG, TQ), np.float64)
        for h in range(H):
            a_h = delta * ap_[h] / NGRID
            b_h = -delta * an_[h] / NGRID
            qfeat[AUG * h : AUG * h + R] = a_h * _ev(psi, fq)
            qfeat[AUG * h + R : AUG * (h + 1)] = b_h * _ev(phi, fq)
        qfeat = qfeat.astype(BF)

        xq = np.asarray(inputs["query"])[b, TQ * p : TQ * (p + 1)]
        xv = np.asarray(inputs["value"])[b, TQ * p : TQ * (p + 1)]
        in_maps.append(
            {
                "xqT": np.ascontiguousarray(xq.T).astype(BF),
                "xkT": np.ascontiguousarray(
                    np.asarray(inputs["key"])[b].T
                ).astype(BF),
                "xvT": np.ascontiguousarray(xv.T).astype(BF),
                "wqT": wqTh,
                "wkP": np.ascontiguousarray(wkT_full[:, perm]).astype(BF),
                "wvT": wvTh,
                "woT": woTh,
                "bq": bqh,
                "bkP": np.ascontiguousarray(bk_full[perm].reshape(2, 128).T),
                "bv": bvh,
                "bo": boh,
                "kfeat": kfeat,
                "qfeat": qfeat,
                "sel2": _state["sel2"],
            }
        )
    return in_maps


def _run(inputs, trace=False, **kw):
    from concourse.bass_utils import run_bass_kernel_spmd

    nc = _build()
    in_maps = _make_in_maps(inputs)
    res = run_bass_kernel_spmd(
        nc, in_maps, core_ids=list(range(NCORES)), trace=trace, **kw
    )
    out = np.zeros((B, T, D), np.float32)
    for c in range(NCORES):
        b, p = c // 4, c % 4
        out[b, TQ * p : TQ * (p + 1)] = res.results[c]["out"]
    return out, res


def kernel(**inputs):
    out, _ = _run(inputs)
    return out
